# revision 5
# baseline (speedup 1.0000x reference)
"""AttnDecoderRNN step on 8 trn2 NeuronCores (Bass/Tile).

Sharding: vocab-parallel embedding table + out projection (6400 padded rows
per core), contraction-sharded LSTM/comb (AllReduce over partial gates),
replicated attention. Collectives: AR-e (4KB), AR-gates (16KB), AG-stats (8B).
Weights stream as bf16; all post-PSUM math fp32.
"""
import numpy as np
import ml_dtypes

H = 1024
V = 50257
L = 12
NCORES = 8
VK = 6400            # per-core padded vocab shard
VPAD = VK * NCORES   # 51200
BF = ml_dtypes.bfloat16

CHUNKS = [(i * 512, 512) for i in range(12)] + [(6144, 256)]


def _build():
    import concourse.bass as bass
    import concourse.tile as tile
    from concourse import bacc, mybir
    from concourse.masks import make_identity

    f32 = mybir.dt.float32
    bf16 = mybir.dt.bfloat16
    i32 = mybir.dt.int32
    AF = mybir.ActivationFunctionType
    ALU = mybir.AluOpType
    AX = mybir.AxisListType
    RG = [list(range(NCORES))]

    nc = bacc.Bacc("TRN2", target_bir_lowering=False, debug=False,
                   num_devices=NCORES)

    # ---- I/O ----
    idx_d = nc.dram_tensor("idx", [1, 1], i32, kind="ExternalInput")
    base_d = nc.dram_tensor("idx_base", [1, 1], i32, kind="ExternalInput")
    emb_d = nc.dram_tensor("embsh", [VK * 8, 128], f32, kind="ExternalInput")
    h0r8_d = nc.dram_tensor("h0r8", [8, 128], f32, kind="ExternalInput")
    h0k_d = nc.dram_tensor("h0k", [1, 128], f32, kind="ExternalInput")
    c0_d = nc.dram_tensor("c0row", [1, H], f32, kind="ExternalInput")
    enc_d = nc.dram_tensor("enc", [L, H], f32, kind="ExternalInput")
    awT_d = nc.dram_tensor("attn_wT", [2 * H, L], f32, kind="ExternalInput")
    ab_d = nc.dram_tensor("attn_b", [1, L], f32, kind="ExternalInput")
    cwT_d = nc.dram_tensor("comb_wT", [2 * H, 128], bf16, kind="ExternalInput")
    cb_d = nc.dram_tensor("comb_b", [1, 128], f32, kind="ExternalInput")
    wih_d = nc.dram_tensor("wihT", [128, 4 * H], bf16, kind="ExternalInput")
    whh_d = nc.dram_tensor("whhT", [128, 4 * H], bf16, kind="ExternalInput")
    bih_d = nc.dram_tensor("bih4", [1, 4 * H], f32, kind="ExternalInput")
    bhh_d = nc.dram_tensor("bhh4", [1, 4 * H], f32, kind="ExternalInput")
    wp_d = nc.dram_tensor("wproj", [8, 128, VK], bf16, kind="ExternalInput")
    bp_d = nc.dram_tensor("bproj", [1, VK], f32, kind="ExternalInput")

    lp_d = nc.dram_tensor("logp", [1, VK], f32, kind="ExternalOutput")
    h_d = nc.dram_tensor("h_out", [1, H], f32, kind="ExternalOutput")
    c_d = nc.dram_tensor("c_out", [1, H], f32, kind="ExternalOutput")
    aw_d = nc.dram_tensor("attn_out", [1, L], f32, kind="ExternalOutput")

    # collective bounce buffers (internal DRAM; outs Shared)
    e_ar_in = nc.dram_tensor("e_ar_in", [8, 128], f32)
    e_ar_out = nc.dram_tensor("e_ar_out", [8, 128], f32, addr_space="Shared")
    g_ar_in = nc.dram_tensor("g_ar_in", [1, 4 * H], f32)
    g_ar_out = nc.dram_tensor("g_ar_out", [1, 4 * H], f32, addr_space="Shared")
    s_ag_in = nc.dram_tensor("s_ag_in", [1, 2], f32)
    s_ag_out = nc.dram_tensor("s_ag_out", [8, 2], f32, addr_space="Shared")

    with tile.TileContext(nc) as tc:
        with tc.tile_pool(name="sb", bufs=1) as sb, \
             tc.tile_pool(name="rows", bufs=5) as rows, \
             tc.tile_pool(name="wl", bufs=3) as wl, \
             tc.tile_pool(name="xs", bufs=2) as xs, \
             tc.tile_pool(name="wp", bufs=1) as wp, \
             tc.tile_pool(name="ps", bufs=4, space="PSUM") as ps:

            ones1 = sb.tile([1, 1], f32)
            nc.gpsimd.memset(ones1[:], 1.0)
            ident8 = sb.tile([8, 8], f32)
            make_identity(nc, ident8[:])

            # ---- projection slabs start streaming immediately ----
            slabs = []
            for k8 in range(8):
                slab = wp.tile([128, VK], bf16, name=f"slab{k8}")
                nc.sync.dma_start(slab[:, 0:VK // 2], wp_d[k8, :, 0:VK // 2])
                nc.sync.dma_start(slab[:, VK // 2:VK], wp_d[k8, :, VK // 2:VK])
                slabs.append(slab)
            logits = sb.tile([1, VK], f32)
            nc.sync.dma_start(logits[:], bp_d[:])   # preload bias

            # ---- embedding gather offsets (device-side) ----
            idx_sb = sb.tile([1, 1], i32)
            nc.gpsimd.dma_start(idx_sb[:], idx_d[:])
            base_sb = sb.tile([1, 1], i32)
            nc.gpsimd.dma_start(base_sb[:], base_d[:])
            d1 = sb.tile([1, 1], i32)
            nc.vector.tensor_tensor(out=d1[:], in0=idx_sb[:], in1=base_sb[:],
                                    op=ALU.subtract)
            d8 = sb.tile([1, 1], i32)
            nc.vector.tensor_scalar_mul(d8[:], d1[:], 8)
            d8c = sb.tile([8, 1], i32)
            for j in range(8):
                nc.gpsimd.dma_start(d8c[j:j + 1, 0:1], d8[0:1, 0:1])
            iota8 = sb.tile([8, 1], i32)
            nc.gpsimd.iota(iota8[:], pattern=[[1, 1]], base=0,
                           channel_multiplier=1)
            offs = sb.tile([8, 1], i32)
            nc.vector.tensor_tensor(out=offs[:], in0=d8c[:], in1=iota8[:],
                                    op=ALU.add)
            offsm = sb.tile([8, 1], i32)
            nc.vector.tensor_scalar(out=offsm[:], in0=offs[:],
                                    scalar1=0x7FFFFFFF, scalar2=None,
                                    op0=ALU.bitwise_and)

            # masked gather: non-owner offsets are OOB -> rows stay zero
            e8 = sb.tile([8, 128], f32)
            nc.gpsimd.memset(e8[:], 0.0)
            nc.gpsimd.indirect_dma_start(
                out=e8[:], out_offset=None,
                in_=emb_d[:],
                in_offset=bass.IndirectOffsetOnAxis(ap=offsm[:, :1], axis=0),
                bounds_check=VK * 8 - 1, oob_is_err=False)

            # AllReduce embedding row
            nc.gpsimd.dma_start(e_ar_in[:], e8[:])
            nc.gpsimd.collective_compute(
                "AllReduce", ALU.add, ins=[e_ar_in.ap().opt()],
                outs=[e_ar_out.ap().opt()], replica_groups=RG)
            e8f = sb.tile([8, 128], f32)
            nc.gpsimd.dma_start(e8f[:], e_ar_out[:])

            # e columns [128,8] (f32 for attention, bf16 for comb)
            pt = ps.tile([128, 8], f32, tag="ps")
            nc.tensor.transpose(out=pt[:], in_=e8f[:], identity=ident8[:])
            e_cols_f = sb.tile([128, 8], f32)
            nc.vector.tensor_copy(e_cols_f[:], pt[:])
            e_cols_b = sb.tile([128, 8], bf16)
            nc.vector.tensor_copy(e_cols_b[:], pt[:])

            # h0 columns
            h8 = sb.tile([8, 128], f32)
            nc.sync.dma_start(h8[:], h0r8_d[:])
            pt2 = ps.tile([128, 8], f32, tag="ps")
            nc.tensor.transpose(out=pt2[:], in_=h8[:], identity=ident8[:])
            h_cols_f = sb.tile([128, 8], f32)
            nc.vector.tensor_copy(h_cols_f[:], pt2[:])

            # ---- attention logits + softmax ----
            awT_sb = sb.tile([128, 16 * L], f32)
            nc.sync.dma_start(
                awT_sb[:].rearrange("p (k j) -> p k j", j=L),
                awT_d.ap().rearrange("(k p) j -> p k j", p=128))
            ab_sb = sb.tile([1, L], f32)
            nc.sync.dma_start(ab_sb[:], ab_d[:])

            pa = ps.tile([1, L], f32, tag="ps")
            for k in range(16):
                cols = e_cols_f if k < 8 else h_cols_f
                nc.tensor.matmul(pa[:], cols[:, (k % 8):(k % 8) + 1],
                                 awT_sb[:, k * L:(k + 1) * L],
                                 start=(k == 0), stop=(k == 15))
            al_sb = sb.tile([1, L], f32)
            nc.vector.tensor_tensor(out=al_sb[:], in0=pa[:], in1=ab_sb[:],
                                    op=ALU.add)
            mx12 = sb.tile([1, 1], f32)
            nc.vector.reduce_max(mx12[:], al_sb[:], axis=AX.X)
            nmx12 = sb.tile([1, 1], f32)
            nc.vector.tensor_scalar_mul(nmx12[:], mx12[:], -1.0)
            ex12 = sb.tile([1, L], f32)
            s12 = sb.tile([1, 1], f32)
            nc.scalar.activation(ex12[:], al_sb[:], AF.Exp,
                                 bias=nmx12[0:1, 0:1], accum_out=s12[:])
            r12 = sb.tile([1, 1], f32)
            nc.vector.reciprocal(r12[:], s12[:])
            aw_sb = sb.tile([1, L], f32)
            nc.vector.tensor_scalar_mul(aw_sb[:], ex12[:], r12[0:1, 0:1])
            nc.sync.dma_start(aw_d[:], aw_sb[:])

            # attn_weights column [12,1]
            paw = ps.tile([L, 1], f32, tag="ps")
            nc.tensor.matmul(paw[:], aw_sb[0:1, 0:L], ones1[0:1, 0:1],
                             start=True, stop=True)
            aw_col = sb.tile([L, 1], f32)
            nc.vector.tensor_copy(aw_col[:], paw[:])

            # attn_applied columns [128,8]
            enc_sb = sb.tile([L, H], f32)
            nc.sync.dma_start(enc_sb[:], enc_d[:])
            pap = ps.tile([128, 8], f32, tag="ps")
            for m in range(8):
                nc.tensor.matmul(pap[:, m:m + 1],
                                 enc_sb[0:L, m * 128:(m + 1) * 128],
                                 aw_col[0:L, 0:1], start=True, stop=True)
            a_cols_b = sb.tile([128, 8], bf16)
            nc.vector.tensor_copy(a_cols_b[:], pap[:])

            # ---- comb (x slice for this core) ----
            cw_sb = sb.tile([128, 16 * 128], bf16)
            nc.sync.dma_start(
                cw_sb[:].rearrange("p (k c) -> p k c", c=128),
                cwT_d.ap().rearrange("(k p) c -> p k c", p=128))
            cb_sb = sb.tile([1, 128], f32)
            nc.sync.dma_start(cb_sb[:], cb_d[:])
            px = ps.tile([1, 128], f32, tag="ps")
            for k in range(16):
                cols = e_cols_b if k < 8 else a_cols_b
                nc.tensor.matmul(px[:], cols[:, (k % 8):(k % 8) + 1],
                                 cw_sb[:, k * 128:(k + 1) * 128],
                                 start=(k == 0), stop=(k == 15))
            xr = sb.tile([1, 128], f32)
            nc.vector.tensor_tensor(out=xr[:], in0=px[:], in1=cb_sb[:],
                                    op=ALU.add)
            nc.scalar.activation(xr[:], xr[:], AF.Relu)

            # x / h0k columns (bf16) via K=1 matmul transpose
            pxc = ps.tile([128, 1], f32, tag="ps")
            nc.tensor.matmul(pxc[:], xr[0:1, 0:128], ones1[0:1, 0:1],
                             start=True, stop=True)
            x_col_b = sb.tile([128, 1], bf16)
            nc.vector.tensor_copy(x_col_b[:], pxc[:])

            h0k_sb = sb.tile([1, 128], f32)
            nc.sync.dma_start(h0k_sb[:], h0k_d[:])
            phk = ps.tile([128, 1], f32, tag="ps")
            nc.tensor.matmul(phk[:], h0k_sb[0:1, 0:128], ones1[0:1, 0:1],
                             start=True, stop=True)
            h0k_col_b = sb.tile([128, 1], bf16)
            nc.vector.tensor_copy(h0k_col_b[:], phk[:])

            # ---- LSTM partial gates (this core's 128-slice of contraction) ----
            # stream W halves [128, 2048] through a 3-slot pool
            wls = {}
            for half in range(2):
                wls[("ih", half)] = wl.tile([128, 2 * H], bf16, tag="wl",
                                            name=f"wih{half}")
                nc.sync.dma_start(wls[("ih", half)][:],
                                  wih_d[:, half * 2 * H:(half + 1) * 2 * H])
                wls[("hh", half)] = wl.tile([128, 2 * H], bf16, tag="wl",
                                            name=f"whh{half}")
                nc.sync.dma_start(wls[("hh", half)][:],
                                  whh_d[:, half * 2 * H:(half + 1) * 2 * H])
            for c in range(8):
                half, loc = c // 4, (c % 4) * 512
                pg = ps.tile([1, 512], f32, tag="ps")
                nc.tensor.matmul(pg[:], x_col_b[:, 0:1],
                                 wls[("ih", half)][:, loc:loc + 512],
                                 start=True, stop=False)
                nc.tensor.matmul(pg[:], h0k_col_b[:, 0:1],
                                 wls[("hh", half)][:, loc:loc + 512],
                                 start=False, stop=True)
                sc = xs.tile([1, 512], f32, tag="xs", name=f"gsc{c}")
                nc.vector.tensor_copy(sc[:], pg[:])
                nc.gpsimd.dma_start(g_ar_in[0:1, c * 512:(c + 1) * 512], sc[:])
            nc.gpsimd.collective_compute(
                "AllReduce", ALU.add, ins=[g_ar_in.ap().opt()],
                outs=[g_ar_out.ap().opt()], replica_groups=RG)

            # ---- pointwise LSTM cell (full, replicated, all on partition 0) ----
            gates_row = sb.tile([1, 4 * H], f32)
            nc.gpsimd.dma_start(gates_row[:], g_ar_out[:])
            for c in range(8):
                sl = slice(c * 512, (c + 1) * 512)
                bsc = xs.tile([1, 512], f32, tag="xs", name=f"bih{c}")
                nc.sync.dma_start(bsc[:], bih_d[0:1, sl])
                nc.vector.tensor_tensor(out=gates_row[0:1, sl],
                                        in0=gates_row[0:1, sl], in1=bsc[:],
                                        op=ALU.add)
                bsc2 = xs.tile([1, 512], f32, tag="xs", name=f"bhh{c}")
                nc.sync.dma_start(bsc2[:], bhh_d[0:1, sl])
                nc.vector.tensor_tensor(out=gates_row[0:1, sl],
                                        in0=gates_row[0:1, sl], in1=bsc2[:],
                                        op=ALU.add)
            nc.scalar.activation(gates_row[0:1, 0:2 * H],
                                 gates_row[0:1, 0:2 * H], AF.Sigmoid)
            nc.scalar.activation(gates_row[0:1, 2 * H:3 * H],
                                 gates_row[0:1, 2 * H:3 * H], AF.Tanh)
            nc.scalar.activation(gates_row[0:1, 3 * H:4 * H],
                                 gates_row[0:1, 3 * H:4 * H], AF.Sigmoid)
            i_v = gates_row[0:1, 0:H]
            f_v = gates_row[0:1, H:2 * H]
            g_v = gates_row[0:1, 2 * H:3 * H]
            o_v = gates_row[0:1, 3 * H:4 * H]
            c0_sb = rows.tile([1, H], f32, tag="row", name="c0sb")
            nc.sync.dma_start(c0_sb[:], c0_d[:])
            t1 = rows.tile([1, H], f32, tag="row", name="t1")
            nc.vector.tensor_tensor(out=t1[:], in0=f_v, in1=c0_sb[:],
                                    op=ALU.mult)             # f*c0
            t2 = rows.tile([1, H], f32, tag="row", name="t2")
            nc.vector.tensor_tensor(out=t2[:], in0=i_v, in1=g_v,
                                    op=ALU.mult)             # i*tanh(g)
            c_new = rows.tile([1, H], f32, tag="row", name="c_new")
            nc.vector.tensor_tensor(out=c_new[:], in0=t1[:], in1=t2[:],
                                    op=ALU.add)
            nc.scalar.activation(t1[:], c_new[:], AF.Tanh)   # tanh(c_new)
            h_new = rows.tile([1, H], f32, tag="row", name="h_new")
            nc.vector.tensor_tensor(out=h_new[:], in0=o_v, in1=t1[:],
                                    op=ALU.mult)
            nc.sync.dma_start(c_d[:], c_new[:])
            nc.sync.dma_start(h_d[:], h_new[:])

            # h_new columns (bf16)
            phc = ps.tile([128, 8], f32, tag="ps")
            for j in range(8):
                nc.tensor.matmul(phc[:, j:j + 1],
                                 h_new[0:1, j * 128:(j + 1) * 128],
                                 ones1[0:1, 0:1], start=True, stop=True)
            hn_cols_b = sb.tile([128, 8], bf16)
            nc.vector.tensor_copy(hn_cols_b[:], phc[:])

            # ---- projection (vocab shard); bias was preloaded into logits ----
            cmx = sb.tile([1, 16], f32)
            for ci, (off, szc) in enumerate(CHUNKS):
                pc = ps.tile([1, 512], f32, tag="ps", name="pc")
                for k8 in range(8):
                    nc.tensor.matmul(pc[0:1, 0:szc],
                                     hn_cols_b[:, k8:k8 + 1],
                                     slabs[k8][:, off:off + szc],
                                     start=(k8 == 0), stop=(k8 == 7))
                nc.vector.tensor_tensor(out=logits[0:1, off:off + szc],
                                        in0=pc[0:1, 0:szc],
                                        in1=logits[0:1, off:off + szc],
                                        op=ALU.add)
                nc.vector.reduce_max(cmx[0:1, ci:ci + 1],
                                     logits[0:1, off:off + szc], axis=AX.X)

            # ---- local log-softmax stats (chunked; no [1,VK] scratch) ----
            mx = sb.tile([1, 1], f32)
            nc.vector.reduce_max(mx[:], cmx[0:1, 0:len(CHUNKS)], axis=AX.X)
            nmx = sb.tile([1, 1], f32)
            nc.vector.tensor_scalar_mul(nmx[:], mx[:], -1.0)
            sums = sb.tile([1, 16], f32)
            for ci, (off, szc) in enumerate(CHUNKS):
                xsc = xs.tile([1, 512], f32, tag="xs", name=f"esc{ci}")
                nc.scalar.activation(xsc[0:1, 0:szc], logits[0:1, off:off + szc],
                                     AF.Exp, bias=nmx[0:1, 0:1],
                                     accum_out=sums[0:1, ci:ci + 1])
            ssum = sb.tile([1, 1], f32)
            nc.vector.reduce_sum(ssum[:], sums[0:1, 0:len(CHUNKS)], axis=AX.X)
            st2 = sb.tile([1, 2], f32)
            nc.vector.tensor_copy(st2[0:1, 0:1], mx[:])
            nc.vector.tensor_copy(st2[0:1, 1:2], ssum[:])
            nc.gpsimd.dma_start(s_ag_in[:], st2[:])
            nc.gpsimd.collective_compute(
                "AllGather", ALU.bypass, ins=[s_ag_in.ap().opt()],
                outs=[s_ag_out.ap().opt()], replica_groups=RG)
            ms_sb = sb.tile([1, 8], f32)
            nc.gpsimd.dma_start(ms_sb[:],
                                s_ag_out.ap().rearrange("r c -> c r")[0:1, :])
            ss_sb = sb.tile([1, 8], f32)
            nc.gpsimd.dma_start(ss_sb[:],
                                s_ag_out.ap().rearrange("r c -> c r")[1:2, :])
            Mg = sb.tile([1, 1], f32)
            nc.vector.reduce_max(Mg[:], ms_sb[:], axis=AX.X)
            nMg = sb.tile([1, 1], f32)
            nc.vector.tensor_scalar_mul(nMg[:], Mg[:], -1.0)
            ea = sb.tile([1, 8], f32)
            nc.scalar.activation(ea[:], ms_sb[:], AF.Exp, bias=nMg[0:1, 0:1])
            sa = sb.tile([1, 8], f32)
            nc.vector.tensor_tensor(out=sa[:], in0=ea[:], in1=ss_sb[:],
                                    op=ALU.mult)
            Sg = sb.tile([1, 1], f32)
            nc.vector.reduce_sum(Sg[:], sa[:], axis=AX.X)
            lg = sb.tile([1, 1], f32)
            nc.scalar.activation(lg[:], Sg[:], AF.Ln)
            logZ = sb.tile([1, 1], f32)
            nc.vector.tensor_tensor(out=logZ[:], in0=lg[:], in1=Mg[:],
                                    op=ALU.add)
            nZ = sb.tile([1, 1], f32)
            nc.vector.tensor_scalar_mul(nZ[:], logZ[:], -1.0)
            nc.vector.tensor_scalar_add(logits[:], logits[:], nZ[0:1, 0:1])
            nc.sync.dma_start(lp_d[:], logits[:])

    nc.compile()
    return nc


def _prep_maps(inputs):
    inp = {k: np.asarray(v) for k, v in inputs.items()}
    emb = inp["emb"].astype(np.float32)
    out_W = inp["out_W"].astype(np.float32)
    out_b = inp["out_b"].astype(np.float32)
    emb_pad = np.zeros((VPAD, H), np.float32)
    emb_pad[:V] = emb
    oW_pad = np.zeros((VPAD, H), np.float32)
    oW_pad[:V] = out_W
    ob_pad = np.full((VPAD,), -1e30, np.float32)
    ob_pad[:V] = out_b

    W_ihT = np.ascontiguousarray(inp["W_ih"].astype(np.float32).T)  # [H, 4H]
    W_hhT = np.ascontiguousarray(inp["W_hh"].astype(np.float32).T)
    comb_W = inp["comb_W"].astype(np.float32)                       # [H, 2H]
    attn_wT = np.ascontiguousarray(inp["attn_W"].astype(np.float32).T)

    h0 = inp["h_hidden"].astype(np.float32).reshape(H)
    c0 = inp["c_hidden"].astype(np.float32).reshape(H)
    idx = inp["input"].reshape(1, 1).astype(np.int32)

    maps = []
    for k in range(NCORES):
        b = k * VK
        s = slice(k * 128, (k + 1) * 128)
        Wk = oW_pad[b:b + VK].astype(BF)                  # [VK, H]
        wproj = np.ascontiguousarray(Wk.T).reshape(8, 128, VK)
        maps.append({
            "idx": idx,
            "idx_base": np.array([[b]], np.int32),
            "embsh": np.ascontiguousarray(emb_pad[b:b + VK]).reshape(VK * 8, 128),
            "h0r8": h0.reshape(8, 128),
            "h0k": h0[s].reshape(1, 128),
            "c0row": c0.reshape(1, H),
            "enc": inp["encoder_outputs"].astype(np.float32),
            "attn_wT": attn_wT,
            "attn_b": inp["attn_b"].astype(np.float32).reshape(1, L),
            "comb_wT": np.ascontiguousarray(comb_W[s, :].T).astype(BF),
            "comb_b": inp["comb_b"].astype(np.float32)[s].reshape(1, 128),
            "wihT": np.ascontiguousarray(W_ihT[s, :]).astype(BF),
            "whhT": np.ascontiguousarray(W_hhT[s, :]).astype(BF),
            "bih4": inp["b_ih"].astype(np.float32).reshape(1, 4 * H),
            "bhh4": inp["b_hh"].astype(np.float32).reshape(1, 4 * H),
            "wproj": wproj,
            "bproj": ob_pad[b:b + VK].reshape(1, VK),
        })
    return maps


_NC = None
_MAPS = None


def kernel(**inputs):
    global _NC, _MAPS
    from concourse import bass_utils
    if _NC is None:
        _NC = _build()
    _MAPS = _prep_maps(inputs)
    res = bass_utils.run_bass_kernel_spmd(_NC, _MAPS, list(range(NCORES)))
    r = res.results
    logp = np.concatenate([r[k]["logp"].reshape(-1) for k in range(NCORES)])
    logp = logp[:V].reshape(1, V).astype(np.float32)
    h_new = r[0]["h_out"].reshape(1, 1, H).astype(np.float32)
    c_new = r[0]["c_out"].reshape(1, 1, H).astype(np.float32)
    attn_w = r[0]["attn_out"].reshape(1, L).astype(np.float32)
    return (logp, (h_new, c_new), attn_w)


# revision 14
# speedup vs baseline: 1.1611x; 1.1611x over previous
"""AttnDecoderRNN step on 8 trn2 NeuronCores (Bass/Tile).

Sharding: vocab-parallel embedding table + out projection (6400 padded rows
per core), contraction-sharded LSTM/comb (AllReduce over partial gates),
replicated attention. Collectives: AR-e (4KB), AR-gates (16KB), AG-stats (8B).
Weights stream as bf16; all post-PSUM math fp32.
"""
import numpy as np
import ml_dtypes

H = 1024
V = 50257
L = 12
NCORES = 8
VK = 6400            # per-core padded vocab shard
VPAD = VK * NCORES   # 51200
BF = ml_dtypes.bfloat16

CHUNKS = [(i * 512, 512) for i in range(12)] + [(6144, 256)]


def _build():
    import concourse.bass as bass
    import concourse.tile as tile
    from concourse import bacc, mybir
    from concourse.masks import make_identity

    f32 = mybir.dt.float32
    bf16 = mybir.dt.bfloat16
    i32 = mybir.dt.int32
    AF = mybir.ActivationFunctionType
    ALU = mybir.AluOpType
    AX = mybir.AxisListType
    RG = [list(range(NCORES))]

    nc = bacc.Bacc("TRN2", target_bir_lowering=False, debug=False,
                   num_devices=NCORES)

    # ---- I/O ----
    idx_d = nc.dram_tensor("idx", [1, 1], i32, kind="ExternalInput")
    bsc_d = nc.dram_tensor("bias_scale", [1, 1], f32, kind="ExternalInput")
    emb_d = nc.dram_tensor("embsh", [V * 8, 128], bf16, kind="ExternalInput")
    h0r8_d = nc.dram_tensor("h0r8", [8, 128], f32, kind="ExternalInput")
    h0k_d = nc.dram_tensor("h0k", [1, 128], f32, kind="ExternalInput")
    c0_d = nc.dram_tensor("c0row", [1, H], f32, kind="ExternalInput")
    enc_d = nc.dram_tensor("enc", [L, H], f32, kind="ExternalInput")
    awT_d = nc.dram_tensor("attn_wT", [2 * H, L], f32, kind="ExternalInput")
    ab_d = nc.dram_tensor("attn_b", [1, L], f32, kind="ExternalInput")
    cwT_d = nc.dram_tensor("comb_wT", [2 * H, 128], bf16, kind="ExternalInput")
    cb_d = nc.dram_tensor("comb_b", [1, 128], f32, kind="ExternalInput")
    wih_d = nc.dram_tensor("wihT", [128, 4 * H], bf16, kind="ExternalInput")
    whh_d = nc.dram_tensor("whhT", [128, 4 * H], bf16, kind="ExternalInput")
    bih_d = nc.dram_tensor("bih4", [1, 4 * H], f32, kind="ExternalInput")
    bhh_d = nc.dram_tensor("bhh4", [1, 4 * H], f32, kind="ExternalInput")
    wp_d = nc.dram_tensor("wproj", [8, 128, VK], bf16, kind="ExternalInput")
    bp_d = nc.dram_tensor("bproj", [1, VK], f32, kind="ExternalInput")

    lp_d = nc.dram_tensor("logp", [1, VK], f32, kind="ExternalOutput")
    h_d = nc.dram_tensor("h_out", [1, H], f32, kind="ExternalOutput")
    c_d = nc.dram_tensor("c_out", [1, H], f32, kind="ExternalOutput")
    aw_d = nc.dram_tensor("attn_out", [1, L], f32, kind="ExternalOutput")

    # collective bounce buffers (internal DRAM; outs Shared)
    g_ar_in = nc.dram_tensor("g_ar_in", [1, 4 * H], f32)
    g_ar_out = nc.dram_tensor("g_ar_out", [1, 4 * H], f32, addr_space="Shared")
    s_ag_in = nc.dram_tensor("s_ag_in", [1, 2], f32)
    s_ag_out = nc.dram_tensor("s_ag_out", [8, 2], f32, addr_space="Shared")

    with tile.TileContext(nc) as tc:
        with tc.tile_pool(name="sb", bufs=1) as sb, \
             tc.tile_pool(name="rows", bufs=5) as rows, \
             tc.tile_pool(name="wl", bufs=3) as wl, \
             tc.tile_pool(name="xs", bufs=4) as xs, \
             tc.tile_pool(name="wp", bufs=1) as wp, \
             tc.tile_pool(name="ps", bufs=4, space="PSUM") as ps:

            ones1 = sb.tile([1, 1], f32)
            nc.gpsimd.memset(ones1[:], 1.0)
            ident8 = sb.tile([8, 8], f32)
            make_identity(nc, ident8[:])
            ident8b = sb.tile([8, 8], bf16)
            make_identity(nc, ident8b[:])

            # ---- embedding gather offsets (device-side) ----
            idx_sb = sb.tile([1, 1], i32)
            nc.gpsimd.dma_start(idx_sb[:], idx_d[:])
            d8 = sb.tile([1, 1], i32)
            nc.vector.tensor_scalar_mul(d8[:], idx_sb[:], 8)
            d8c = sb.tile([8, 1], i32)
            for j in range(8):
                nc.gpsimd.dma_start(d8c[j:j + 1, 0:1], d8[0:1, 0:1])
            iota8 = sb.tile([8, 1], i32)
            nc.gpsimd.iota(iota8[:], pattern=[[1, 1]], base=0,
                           channel_multiplier=1)
            offs = sb.tile([8, 1], i32)
            nc.vector.tensor_tensor(out=offs[:], in0=d8c[:], in1=iota8[:],
                                    op=ALU.add)

            # local gather from the (replicated) bf16 table
            e8 = sb.tile([8, 128], bf16)
            nc.gpsimd.indirect_dma_start(
                out=e8[:], out_offset=None,
                in_=emb_d[:],
                in_offset=bass.IndirectOffsetOnAxis(ap=offs[:, :1], axis=0),
                bounds_check=V * 8 - 1, oob_is_err=False)

            # e columns [128,8] (f32 for attention, bf16 for comb)
            pt = ps.tile([128, 8], bf16, tag="ps")
            nc.tensor.transpose(out=pt[:], in_=e8[:], identity=ident8b[:])
            e_cols_f = sb.tile([128, 8], f32)
            nc.vector.tensor_copy(e_cols_f[:], pt[:])
            e_cols_b = sb.tile([128, 8], bf16)
            nc.vector.tensor_copy(e_cols_b[:], pt[:])

            # h0 columns
            h8 = sb.tile([8, 128], f32)
            nc.sync.dma_start(h8[:], h0r8_d[:])
            pt2 = ps.tile([128, 8], f32, tag="ps")
            nc.tensor.transpose(out=pt2[:], in_=h8[:], identity=ident8[:])
            h_cols_f = sb.tile([128, 8], f32)
            nc.vector.tensor_copy(h_cols_f[:], pt2[:])

            # ---- attention logits + softmax ----
            awT_sb = sb.tile([128, 16 * L], f32)
            nc.sync.dma_start(
                awT_sb[:].rearrange("p (k j) -> p k j", j=L),
                awT_d.ap().rearrange("(k p) j -> p k j", p=128))
            ab_sb = sb.tile([1, L], f32)
            nc.sync.dma_start(ab_sb[:], ab_d[:])

            pa = ps.tile([1, L], f32, tag="ps")
            for k in range(16):
                cols = e_cols_f if k < 8 else h_cols_f
                nc.tensor.matmul(pa[:], cols[:, (k % 8):(k % 8) + 1],
                                 awT_sb[:, k * L:(k + 1) * L],
                                 start=(k == 0), stop=(k == 15))
            al_sb = sb.tile([1, L], f32)
            nc.vector.tensor_tensor(out=al_sb[:], in0=pa[:], in1=ab_sb[:],
                                    op=ALU.add)
            mx12 = sb.tile([1, 1], f32)
            nc.vector.reduce_max(mx12[:], al_sb[:], axis=AX.X)
            nmx12 = sb.tile([1, 1], f32)
            nc.vector.tensor_scalar_mul(nmx12[:], mx12[:], -1.0)
            ex12 = sb.tile([1, L], f32)
            s12 = sb.tile([1, 1], f32)
            nc.scalar.activation(ex12[:], al_sb[:], AF.Exp,
                                 bias=nmx12[0:1, 0:1], accum_out=s12[:])
            r12 = sb.tile([1, 1], f32)
            nc.vector.reciprocal(r12[:], s12[:])
            aw_sb = sb.tile([1, L], f32)
            nc.vector.tensor_scalar_mul(aw_sb[:], ex12[:], r12[0:1, 0:1])
            nc.sync.dma_start(aw_d[:], aw_sb[:])

            # attn_weights column [12,1]
            paw = ps.tile([L, 1], f32, tag="ps")
            nc.tensor.matmul(paw[:], aw_sb[0:1, 0:L], ones1[0:1, 0:1],
                             start=True, stop=True)
            aw_col = sb.tile([L, 1], f32)
            nc.vector.tensor_copy(aw_col[:], paw[:])

            # attn_applied columns [128,8]
            enc_sb = sb.tile([L, H], f32)
            nc.sync.dma_start(enc_sb[:], enc_d[:])
            pap = ps.tile([128, 8], f32, tag="ps")
            for m in range(8):
                nc.tensor.matmul(pap[:, m:m + 1],
                                 enc_sb[0:L, m * 128:(m + 1) * 128],
                                 aw_col[0:L, 0:1], start=True, stop=True)
            a_cols_b = sb.tile([128, 8], bf16)
            nc.vector.tensor_copy(a_cols_b[:], pap[:])

            # ---- comb (x slice for this core) ----
            cw_sb = sb.tile([128, 16 * 128], bf16)
            nc.sync.dma_start(
                cw_sb[:].rearrange("p (k c) -> p k c", c=128),
                cwT_d.ap().rearrange("(k p) c -> p k c", p=128))
            cb_sb = sb.tile([1, 128], f32)
            nc.sync.dma_start(cb_sb[:], cb_d[:])
            px = ps.tile([1, 128], f32, tag="ps")
            for k in range(16):
                cols = e_cols_b if k < 8 else a_cols_b
                nc.tensor.matmul(px[:], cols[:, (k % 8):(k % 8) + 1],
                                 cw_sb[:, k * 128:(k + 1) * 128],
                                 start=(k == 0), stop=(k == 15))
            xr = sb.tile([1, 128], f32)
            nc.vector.tensor_tensor(out=xr[:], in0=px[:], in1=cb_sb[:],
                                    op=ALU.add)
            nc.scalar.activation(xr[:], xr[:], AF.Relu)

            # x / h0k columns (bf16) via K=1 matmul transpose
            pxc = ps.tile([128, 1], f32, tag="ps")
            nc.tensor.matmul(pxc[:], xr[0:1, 0:128], ones1[0:1, 0:1],
                             start=True, stop=True)
            x_col_b = sb.tile([128, 1], bf16)
            nc.vector.tensor_copy(x_col_b[:], pxc[:])

            h0k_sb = sb.tile([1, 128], f32)
            nc.sync.dma_start(h0k_sb[:], h0k_d[:])
            phk = ps.tile([128, 1], f32, tag="ps")
            nc.tensor.matmul(phk[:], h0k_sb[0:1, 0:128], ones1[0:1, 0:1],
                             start=True, stop=True)
            h0k_col_b = sb.tile([128, 1], bf16)
            nc.vector.tensor_copy(h0k_col_b[:], phk[:])

            # ---- LSTM partial gates (this core's 128-slice of contraction) ----
            # stream W halves [128, 2048] through a 3-slot pool
            wls = {}
            for half in range(2):
                wls[("ih", half)] = wl.tile([128, 2 * H], bf16, tag="wl",
                                            name=f"wih{half}")
                nc.sync.dma_start(wls[("ih", half)][:],
                                  wih_d[:, half * 2 * H:(half + 1) * 2 * H])
                wls[("hh", half)] = wl.tile([128, 2 * H], bf16, tag="wl",
                                            name=f"whh{half}")
                nc.sync.dma_start(wls[("hh", half)][:],
                                  whh_d[:, half * 2 * H:(half + 1) * 2 * H])
            bscale = sb.tile([1, 1], f32)
            nc.sync.dma_start(bscale[:], bsc_d[:])
            for c in range(8):
                half, loc = c // 4, (c % 4) * 512
                sl = slice(c * 512, (c + 1) * 512)
                pg = ps.tile([1, 512], f32, tag="ps")
                nc.tensor.matmul(pg[:], x_col_b[:, 0:1],
                                 wls[("ih", half)][:, loc:loc + 512],
                                 start=True, stop=False)
                nc.tensor.matmul(pg[:], h0k_col_b[:, 0:1],
                                 wls[("hh", half)][:, loc:loc + 512],
                                 start=False, stop=True)
                # fold (b_ih+b_hh)*bias_scale into the partial pre-AllReduce
                # (bias_scale is 1.0 on core 0 only)
                bb1 = xs.tile([1, 512], f32, tag="xs", name=f"bb1{c}")
                nc.sync.dma_start(bb1[:], bih_d[0:1, sl])
                bb2 = xs.tile([1, 512], f32, tag="xs", name=f"bb2{c}")
                nc.sync.dma_start(bb2[:], bhh_d[0:1, sl])
                nc.vector.tensor_tensor(out=bb1[:], in0=bb1[:], in1=bb2[:],
                                        op=ALU.add)
                nc.vector.tensor_scalar_mul(bb1[:], bb1[:], bscale[0:1, 0:1])
                sc = xs.tile([1, 512], f32, tag="xs", name=f"gsc{c}")
                nc.vector.tensor_tensor(out=sc[:], in0=pg[:], in1=bb1[:],
                                        op=ALU.add)
                nc.gpsimd.dma_start(g_ar_in[0:1, sl], sc[:])
            nc.gpsimd.collective_compute(
                "AllReduce", ALU.add, ins=[g_ar_in.ap().opt()],
                outs=[g_ar_out.ap().opt()], replica_groups=RG)

            # ---- pointwise LSTM cell (full, replicated, all on partition 0) ----
            gates_row = sb.tile([1, 4 * H], f32)
            nc.gpsimd.dma_start(gates_row[:], g_ar_out[:])
            nc.scalar.activation(gates_row[0:1, 0:2 * H],
                                 gates_row[0:1, 0:2 * H], AF.Sigmoid)
            nc.scalar.activation(gates_row[0:1, 2 * H:3 * H],
                                 gates_row[0:1, 2 * H:3 * H], AF.Tanh)
            nc.scalar.activation(gates_row[0:1, 3 * H:4 * H],
                                 gates_row[0:1, 3 * H:4 * H], AF.Sigmoid)
            i_v = gates_row[0:1, 0:H]
            f_v = gates_row[0:1, H:2 * H]
            g_v = gates_row[0:1, 2 * H:3 * H]
            o_v = gates_row[0:1, 3 * H:4 * H]
            c0_sb = rows.tile([1, H], f32, tag="row", name="c0sb")
            nc.sync.dma_start(c0_sb[:], c0_d[:])
            t1 = rows.tile([1, H], f32, tag="row", name="t1")
            nc.vector.tensor_tensor(out=t1[:], in0=f_v, in1=c0_sb[:],
                                    op=ALU.mult)             # f*c0
            t2 = rows.tile([1, H], f32, tag="row", name="t2")
            nc.vector.tensor_tensor(out=t2[:], in0=i_v, in1=g_v,
                                    op=ALU.mult)             # i*tanh(g)
            c_new = rows.tile([1, H], f32, tag="row", name="c_new")
            nc.vector.tensor_tensor(out=c_new[:], in0=t1[:], in1=t2[:],
                                    op=ALU.add)
            nc.scalar.activation(t1[:], c_new[:], AF.Tanh)   # tanh(c_new)
            h_new = rows.tile([1, H], f32, tag="row", name="h_new")
            nc.vector.tensor_tensor(out=h_new[:], in0=o_v, in1=t1[:],
                                    op=ALU.mult)
            nc.sync.dma_start(c_d[:], c_new[:])
            nc.sync.dma_start(h_d[:], h_new[:])

            # h_new columns (bf16)
            phc = ps.tile([128, 8], f32, tag="ps")
            for j in range(8):
                nc.tensor.matmul(phc[:, j:j + 1],
                                 h_new[0:1, j * 128:(j + 1) * 128],
                                 ones1[0:1, 0:1], start=True, stop=True)
            hn_cols_b = sb.tile([128, 8], bf16)
            nc.vector.tensor_copy(hn_cols_b[:], phc[:])

            # ---- projection slabs: emitted late = low scheduler priority, so
            # the serial pre-AllReduce chain's small DMAs never queue behind
            # bulk pieces; fine-grained so any head-of-line block is short ----
            slabs = []
            PIECE = 800
            for k8 in range(8):
                slab = wp.tile([128, VK], bf16, name=f"slab{k8}")
                for p0 in range(0, VK, PIECE):
                    nc.sync.dma_start(slab[:, p0:p0 + PIECE],
                                      wp_d[k8, :, p0:p0 + PIECE])
                slabs.append(slab)
            logits = sb.tile([1, VK], f32)
            nc.sync.dma_start(logits[:], bp_d[:])   # preload bias

            # ---- projection (vocab shard); bias was preloaded into logits ----
            cmx = sb.tile([1, 16], f32)
            for ci, (off, szc) in enumerate(CHUNKS):
                pc = ps.tile([1, 512], f32, tag="ps", name="pc")
                for k8 in range(8):
                    nc.tensor.matmul(pc[0:1, 0:szc],
                                     hn_cols_b[:, k8:k8 + 1],
                                     slabs[k8][:, off:off + szc],
                                     start=(k8 == 0), stop=(k8 == 7))
                nc.vector.tensor_tensor(out=logits[0:1, off:off + szc],
                                        in0=pc[0:1, 0:szc],
                                        in1=logits[0:1, off:off + szc],
                                        op=ALU.add)
                nc.vector.reduce_max(cmx[0:1, ci:ci + 1],
                                     logits[0:1, off:off + szc], axis=AX.X)

            # ---- local log-softmax stats (chunked; no [1,VK] scratch) ----
            mx = sb.tile([1, 1], f32)
            nc.vector.reduce_max(mx[:], cmx[0:1, 0:len(CHUNKS)], axis=AX.X)
            nmx = sb.tile([1, 1], f32)
            nc.vector.tensor_scalar_mul(nmx[:], mx[:], -1.0)
            sums = sb.tile([1, 16], f32)
            for ci, (off, szc) in enumerate(CHUNKS):
                xsc = xs.tile([1, 512], f32, tag="xs", name=f"esc{ci}")
                nc.scalar.activation(xsc[0:1, 0:szc], logits[0:1, off:off + szc],
                                     AF.Exp, bias=nmx[0:1, 0:1],
                                     accum_out=sums[0:1, ci:ci + 1])
            ssum = sb.tile([1, 1], f32)
            nc.vector.reduce_sum(ssum[:], sums[0:1, 0:len(CHUNKS)], axis=AX.X)
            st2 = sb.tile([1, 2], f32)
            nc.vector.tensor_copy(st2[0:1, 0:1], mx[:])
            nc.vector.tensor_copy(st2[0:1, 1:2], ssum[:])
            nc.gpsimd.dma_start(s_ag_in[:], st2[:])
            nc.gpsimd.collective_compute(
                "AllGather", ALU.bypass, ins=[s_ag_in.ap().opt()],
                outs=[s_ag_out.ap().opt()], replica_groups=RG)
            ms_sb = sb.tile([1, 8], f32)
            nc.gpsimd.dma_start(ms_sb[:],
                                s_ag_out.ap().rearrange("r c -> c r")[0:1, :])
            ss_sb = sb.tile([1, 8], f32)
            nc.gpsimd.dma_start(ss_sb[:],
                                s_ag_out.ap().rearrange("r c -> c r")[1:2, :])
            Mg = sb.tile([1, 1], f32)
            nc.vector.reduce_max(Mg[:], ms_sb[:], axis=AX.X)
            nMg = sb.tile([1, 1], f32)
            nc.vector.tensor_scalar_mul(nMg[:], Mg[:], -1.0)
            ea = sb.tile([1, 8], f32)
            nc.scalar.activation(ea[:], ms_sb[:], AF.Exp, bias=nMg[0:1, 0:1])
            sa = sb.tile([1, 8], f32)
            nc.vector.tensor_tensor(out=sa[:], in0=ea[:], in1=ss_sb[:],
                                    op=ALU.mult)
            Sg = sb.tile([1, 1], f32)
            nc.vector.reduce_sum(Sg[:], sa[:], axis=AX.X)
            lg = sb.tile([1, 1], f32)
            nc.scalar.activation(lg[:], Sg[:], AF.Ln)
            logZ = sb.tile([1, 1], f32)
            nc.vector.tensor_tensor(out=logZ[:], in0=lg[:], in1=Mg[:],
                                    op=ALU.add)
            nZ = sb.tile([1, 1], f32)
            nc.vector.tensor_scalar_mul(nZ[:], logZ[:], -1.0)
            nc.vector.tensor_scalar_add(logits[:], logits[:], nZ[0:1, 0:1])
            nc.sync.dma_start(lp_d[:], logits[:])

    nc.compile()
    return nc


def _prep_maps(inputs):
    inp = {k: np.asarray(v) for k, v in inputs.items()}
    emb_bf = np.ascontiguousarray(inp["emb"].astype(BF)).reshape(V * 8, 128)
    out_W = inp["out_W"].astype(np.float32)
    out_b = inp["out_b"].astype(np.float32)
    oW_pad = np.zeros((VPAD, H), np.float32)
    oW_pad[:V] = out_W
    ob_pad = np.full((VPAD,), -1e30, np.float32)
    ob_pad[:V] = out_b

    W_ihT = np.ascontiguousarray(inp["W_ih"].astype(np.float32).T)  # [H, 4H]
    W_hhT = np.ascontiguousarray(inp["W_hh"].astype(np.float32).T)
    comb_W = inp["comb_W"].astype(np.float32)                       # [H, 2H]
    attn_wT = np.ascontiguousarray(inp["attn_W"].astype(np.float32).T)

    h0 = inp["h_hidden"].astype(np.float32).reshape(H)
    c0 = inp["c_hidden"].astype(np.float32).reshape(H)
    idx = inp["input"].reshape(1, 1).astype(np.int32)

    maps = []
    for k in range(NCORES):
        b = k * VK
        s = slice(k * 128, (k + 1) * 128)
        Wk = oW_pad[b:b + VK].astype(BF)                  # [VK, H]
        wproj = np.ascontiguousarray(Wk.T).reshape(8, 128, VK)
        maps.append({
            "idx": idx,
            "bias_scale": np.array([[1.0 if k == 0 else 0.0]], np.float32),
            "embsh": emb_bf,
            "h0r8": h0.reshape(8, 128),
            "h0k": h0[s].reshape(1, 128),
            "c0row": c0.reshape(1, H),
            "enc": inp["encoder_outputs"].astype(np.float32),
            "attn_wT": attn_wT,
            "attn_b": inp["attn_b"].astype(np.float32).reshape(1, L),
            "comb_wT": np.ascontiguousarray(comb_W[s, :].T).astype(BF),
            "comb_b": inp["comb_b"].astype(np.float32)[s].reshape(1, 128),
            "wihT": np.ascontiguousarray(W_ihT[s, :]).astype(BF),
            "whhT": np.ascontiguousarray(W_hhT[s, :]).astype(BF),
            "bih4": inp["b_ih"].astype(np.float32).reshape(1, 4 * H),
            "bhh4": inp["b_hh"].astype(np.float32).reshape(1, 4 * H),
            "wproj": wproj,
            "bproj": ob_pad[b:b + VK].reshape(1, VK),
        })
    return maps


_NC = None
_MAPS = None


def kernel(**inputs):
    global _NC, _MAPS
    from concourse import bass_utils
    if _NC is None:
        _NC = _build()
    _MAPS = _prep_maps(inputs)
    res = bass_utils.run_bass_kernel_spmd(_NC, _MAPS, list(range(NCORES)))
    r = res.results
    logp = np.concatenate([r[k]["logp"].reshape(-1) for k in range(NCORES)])
    logp = logp[:V].reshape(1, V).astype(np.float32)
    h_new = r[0]["h_out"].reshape(1, 1, H).astype(np.float32)
    c_new = r[0]["c_out"].reshape(1, 1, H).astype(np.float32)
    attn_w = r[0]["attn_out"].reshape(1, L).astype(np.float32)
    return (logp, (h_new, c_new), attn_w)
